# revision 55
# baseline (speedup 1.0000x reference)
"""Multi-head attention (B=2, S=2048, nx=768, H=12) on 8 TRN2 NeuronCores.

Sharding: 24 (batch, head) pairs -> 3 heads per core. Core c handles batch
c//4, heads {3*(c%4), +1, +2}. Each core computes QKV projection for its
head slice, attention, and a partial output projection (its 192 rows of
w_proj); the host sums the 4 partials per batch and adds b_proj.

v2 schedule (ACT-exp is the ~107us floor; everything else overlaps it):
  - chunked priority DMA: wqk first, then xt by 512-token chunk, so
    qk_proj(0) starts ~2us in and the first exp lands ~8us in.
  - scores psum tiles are [128, 1024] = (even t-chunk 512q | odd 512q);
    ONE exp per tile means both matmuls of the next tile wait on the same
    semaphore and run CONCURRENTLY as PE row-tiles (rows 0:64 even chunks,
    64:128 odd) - measured 132 ns/MM vs 261 serial.
  - v_proj + qk_proj(1,2) are emitted as PE filler inside the head-0
    scores window; the per-head loop interleaves PV(h) with scores(h+1).
  - KCH=6: the zero-bias fast path drops the bias/pad contraction chunk
    (b_attn is zeros per spec); a KCH=7 bias build is compiled on demand.
  - tail: proj K=128 (heads 0,1) matmuls pre-issued during head-2's PV,
    norm staging for head 2 on ACT (idle post-exp), the 4 reciprocal-
    broadcast matmuls issued back-to-back at row positions 0/32/64/96
    (concurrent), output staged to bf16 and summed on host in f32.
"""

import numpy as np
import ml_dtypes

import concourse.bass as bass
import concourse.tile as tile
import concourse.mybir as mybir
from concourse import bacc

BF16 = mybir.dt.bfloat16
F32 = mybir.dt.float32

NX = 768
D = 64
HPC = 3          # heads per core
N_CORES = 8


def build_nc(S=2048, use_bias=False):
    """Build the single-core SPMD program."""
    KCH = 7 if use_bias else 6   # contraction chunks of 128
    KDIM = KCH * 128
    nc = bacc.Bacc("TRN2", target_bir_lowering=False, debug=False)

    # xt is partition-major on the host: [p, qc, c, tok] so each per-qc DMA
    # reads 6KB-contiguous per partition (descriptor-efficient).
    xt_d = nc.dram_tensor("xt", [128, S // 512, KCH, 512], BF16,
                          kind="ExternalInput")
    wqk_d = nc.dram_tensor("wqk", [KDIM, 6 * D], BF16, kind="ExternalInput")
    wv_d = nc.dram_tensor("wv", [KDIM, HPC * D], BF16, kind="ExternalInput")
    wp_d = nc.dram_tensor("wp", [HPC * D, NX], BF16, kind="ExternalInput")
    # two partial outputs: head-0's projection (runs during windows 1-2)
    # and heads 1+2 (tail); the host sums them
    outs_d = [nc.dram_tensor(f"out{h}", [S, NX], BF16, kind="ExternalOutput")
              for h in range(2)]

    with tile.TileContext(nc) as tc:
        _build_body(tc, [o.ap() for o in outs_d], xt_d.ap(), wqk_d.ap(),
                    wv_d.ap(), wp_d.ap(), S, KCH)
    nc.compile()
    return nc


def _build_body(tc, outs_d, xt_d, wqk_d, wv_d, wp_d, S, KCH):
    nc = tc.nc
    P = 128
    TC = S // 128    # t (key) chunks
    QC = S // 512    # q chunks of 512
    NPAIR = TC // 2  # t-chunk pairs per head

    with tc.tile_pool(name="const", bufs=1) as cpool, \
         tc.tile_pool(name="epool", bufs=NPAIR + 2) as epool, \
         tc.tile_pool(name="small", bufs=4) as spool, \
         tc.tile_pool(name="ostage", bufs=3) as opool, \
         tc.tile_pool(name="pvpool", bufs=QC + 1) as pvpool, \
         tc.tile_pool(name="ps_score", bufs=2, space="PSUM") as ps_score, \
         tc.tile_pool(name="ps_pv", bufs=QC, space="PSUM") as ps_pv:

        # ---- stage inputs in SBUF (priority order: wqk, xt chunks, ...) ----
        wqk_sb = cpool.tile([P, KCH, 6 * D], BF16)
        wqk_r = wqk_d.rearrange("(c p) m -> p c m", p=P)
        nc.sync.dma_start(wqk_sb[:, :, 0:128], wqk_r[:, :, 0:128])
        xt_sb = cpool.tile([P, KCH, S], BF16)
        for qc in range(QC):
            sl = slice(qc * 512, (qc + 1) * 512)
            nc.sync.dma_start(xt_sb[:, :, sl], xt_d[:, qc, :, :])
        nc.sync.dma_start(wqk_sb[:, :, 128:384], wqk_r[:, :, 128:384])
        wv_sb = cpool.tile([P, KCH, HPC * D], BF16)
        nc.sync.dma_start(wv_sb[:], wv_d.rearrange("(c p) m -> p c m", p=P))
        wp0_sb = cpool.tile([D, NX], BF16)
        nc.sync.dma_start(wp0_sb[:], wp_d[0:D, :])
        wp12_sb = cpool.tile([P, NX], BF16)
        nc.sync.dma_start(wp12_sb[:], wp_d[D:HPC * D, :])
        ones4 = cpool.tile([97, D], F32)
        nc.vector.memset(ones4[:], 1.0)

        # q2: Q^T duplicated into both partition halves (rows 0:64 == 64:128)
        # k2: K^T with even token-chunks in rows 0:64, odd in rows 64:128 --
        # the stationary layout for the row-paired scores matmuls.
        q2_sb = cpool.tile([P, HPC, S], BF16)
        k2_sb = cpool.tile([P, HPC, S // 2], BF16)
        v_sb = cpool.tile([P, TC, HPC, D + 1], BF16)
        aT_0 = cpool.tile([D, S], BF16)    # head 0 (projected during w1-w2)
        aT_bc = cpool.tile([P, S], BF16)   # heads 1,2 stacked (tail proj)

        # wqk col order is [qA kA qB kB qC kC]; m-chunk mc covers head mc's
        # q (psum partitions 0:64) and k (64:128). Transient psums all come
        # from ps_score so ps_pv holds only the long-lived PV accumulators.
        def qk_proj(mc, qcs, dup_now=False):
            for qc in qcs:
                ps = ps_score.tile([P, 1024], F32, tag="score",
                                   name=f"qk_{mc}_{qc}")
                for kc in range(KCH):
                    nc.tensor.matmul(
                        ps[:, 0:512],
                        wqk_sb[:, kc, mc * 128:(mc + 1) * 128],
                        xt_sb[:, kc, qc * 512:(qc + 1) * 512],
                        start=(kc == 0), stop=(kc == KCH - 1))
                nc.vector.tensor_copy(q2_sb[0:D, mc, qc * 512:(qc + 1) * 512],
                                      ps[0:D, 0:512])
                kview = ps[D:P, 0:512].rearrange("p (b c) -> p b c", c=128)
                k2w = k2_sb[:, mc, qc * 256:(qc + 1) * 256].rearrange(
                    "p (b c) -> p b c", c=128)
                nc.vector.tensor_copy(k2w[0:D], kview[:, 0::2, :])
                nc.vector.tensor_copy(k2w[D:P], kview[:, 1::2, :])
                if dup_now:
                    nc.vector.tensor_copy(
                        q2_sb[D:P, mc, qc * 512:(qc + 1) * 512],
                        q2_sb[0:D, mc, qc * 512:(qc + 1) * 512])
            if not dup_now and qcs[-1] == QC - 1:
                # duplicate q into the lower half (DVE 4x bf16 copy)
                nc.vector.tensor_copy(q2_sb[D:P, mc, :], q2_sb[0:D, mc, :])

        def v_proj(ts):
            if ts[0] == 0:
                nc.vector.memset(v_sb[:, :, :, D:D + 1], 1.0)
            for t in ts:
                ps = ps_score.tile([P, 1024], F32, tag="score", name=f"v_{t}")
                for kc in range(KCH):
                    nc.tensor.matmul(
                        ps[:, 0:HPC * D],
                        xt_sb[:, kc, t * 128:(t + 1) * 128],
                        wv_sb[:, kc, :],
                        start=(kc == 0), stop=(kc == KCH - 1))
                nc.vector.tensor_copy(
                    v_sb[:, t, :, 0:D],
                    ps[:, 0:HPC * D].rearrange("p (h d) -> p h d", h=HPC))

        e_tiles = {}
        warm_state = {"pvs": None, "i": 0}

        def warm_pulse(e2, qq):
            # HAM warmkeeper: a K=1 N=512 matmul (~213ns) fired right after
            # each exp, targeting unused partitions 65+ of a live PV
            # accumulator bank, so the PE never sits a full MID window idle.
            pvs = warm_state["pvs"]
            if pvs is None:
                return
            i = warm_state["i"] = (warm_state["i"] + 1) % QC
            nc.tensor.matmul(pvs[i][96:97, :],
                             e2[0:1, qq, 0, 0:1], e2[0:1, qq, 0, :],
                             start=True, stop=True, skip_group_check=True,
                             tile_position=(0, 96))

        def scores_exp_pair(h, j, qqs=None):
            # t-chunks (2j, 2j+1) as concurrent PE row-tiles: even chunk from
            # k2 rows 0:64 -> psum cols 0:512, odd from rows 64:128 -> cols
            # 512:1024. One exp per psum tile keeps the pair on one sem, so
            # the two matmuls issue back-to-back and overlap in the array.
            # e2 dims [qq, parity, tok] keep the exp output AP contiguous.
            if qqs is None:
                qqs = range(QC)
            if (h, 2 * j) in e_tiles:
                e2 = e_tiles[(h, 2 * j)]
            else:
                e2 = epool.tile([P, QC, 2, 512], BF16, tag="E",
                                name=f"e_{h}_{j}")
                e_tiles[(h, 2 * j)] = e2
            for qq in qqs:
                ps = ps_score.tile([P, 1024], F32, tag="score", name="sc")
                qsl = slice(qq * 512, (qq + 1) * 512)
                nc.tensor.matmul(
                    ps[:, 0:512],
                    k2_sb[0:D, h, j * 128:(j + 1) * 128],
                    q2_sb[0:D, h, qsl], start=True, stop=True)
                nc.tensor.matmul(
                    ps[:, 512:1024],
                    k2_sb[D:P, h, j * 128:(j + 1) * 128],
                    q2_sb[D:P, h, qsl], start=True, stop=True)
                nc.scalar.activation(
                    e2[:, qq, :, :], ps[:],
                    mybir.ActivationFunctionType.Exp, scale=0.125)
                warm_pulse(e2, qq)

        def pv_chunk(h, t, pvs):
            e2 = e_tiles[(h, t & ~1)]
            for qc in range(QC):
                nc.tensor.matmul(
                    pvs[qc][0:D + 1, :],
                    v_sb[:, t, h, :],
                    e2[:, qc, t & 1, :],
                    start=(t == 0), stop=(t == TC - 1))
            if t & 1:
                e_tiles.pop((h, t & ~1))

        # sumexp rows gathered at 32-aligned partitions -> one batched
        # reciprocal; 1/Z broadcast via K=1 matmuls at row positions
        # 0/32/64/96. Stage a (copies + reciprocal) releases the PV
        # accumulators; stage b (rb matmuls + mults) is emitted two pairs
        # later so the PE never FIFO-blocks on the reciprocal. Head 2's
        # staging runs on both ACT (idle after the last exp) and DVE.
        def norm_stage_a(h, pvs):
            if h == HPC - 1:
                def stage_copy(dst, src, qc):
                    if qc < 2:
                        nc.vector.tensor_copy(dst, src)
                    else:
                        nc.scalar.copy(dst, src)
            else:
                def stage_copy(dst, src, qc):
                    nc.vector.tensor_copy(dst, src)
            rt = spool.tile([97, 512], F32, tag="rt")
            nc.vector.memset(rt[:], 1.0)
            pvsbs = []
            for qc in range(QC):
                pvsb = pvpool.tile([D, 512], F32, tag="pvsb",
                                   name=f"pvsb_{h}_{qc}")
                stage_copy(pvsb[:], pvs[qc][0:D, :], qc)
                stage_copy(rt[32 * qc:32 * qc + 1, :],
                           pvs[qc][D:D + 1, :], qc)
                pvsbs.append(pvsb)
            rr = spool.tile([97, 512], F32, tag="rr")
            if h == HPC - 1:
                # split so warmkeeper matmuls can stagger through it
                nc.vector.reciprocal(rr[:, 0:256], rt[:, 0:256])
                nc.vector.reciprocal(rr[:, 256:512], rt[:, 256:512])
            else:
                nc.vector.reciprocal(rr[:], rt[:])
            return rr, pvsbs

        def norm_stage_b(h, rr, pvsbs, pool=None):
            for qc in range(QC):
                if pool is None:
                    rb = ps_score.tile([P, 1024], F32, tag="score",
                                       name=f"rb_{h}_{qc}")
                else:
                    rb = pool.tile([P, 512], F32, tag="pv",
                                   name=f"rb_{h}_{qc}")
                nc.tensor.matmul(rb[0:D, 0:512],
                                 ones4[32 * qc:32 * qc + 1, :],
                                 rr[32 * qc:32 * qc + 1, :],
                                 start=True, stop=True,
                                 tile_position=(32 * qc, 0))
                qsl = slice(qc * 512, (qc + 1) * 512)
                dst = (aT_0[:, qsl] if h == 0
                       else aT_bc[(h - 1) * D:h * D, qsl])
                nc.vector.tensor_tensor(dst, pvsbs[qc][:], rb[0:D, 0:512],
                                        mybir.AluOpType.mult)

        # Per-head output projection: head hh's 64 aT rows x its 64 rows of
        # w_proj (K=64 matmuls). Head 0 runs during window 1, head 1 during
        # late window 2 (both only need their own norm), head 2 in the tail.
        # Staging converts to bf16; 4 chunks batch into one DMA issue.
        ostage_cur = {}

        def proj_head(hh, sc, use_act=False):
            # hh=0: head-0 rows only (runs during w1, stream out0)
            # hh=2: heads 1+2 accumulated in one psum (tail, stream out1)
            s_sl = slice(sc * 128, (sc + 1) * 128)
            if hh == 2 and sc % 4 >= 2:
                ta = ps_pv.tile([P, 512], F32, tag="pv", name=f"pja_{sc}")
                tb = ps_pv.tile([P, 512], F32, tag="pv", name=f"pjb_{sc}")
                pa, pb = ta[:], tb[:, 0:256]
            else:
                ps = ps_score.tile([P, 1024], F32, tag="score",
                                   name=f"pj_{hh}_{sc}")
                pa, pb = ps[:, 0:512], ps[:, 512:768]
            if hh == 0:
                nc.tensor.matmul(pa, aT_0[:, s_sl], wp0_sb[:, 0:512],
                                 start=True, stop=True)
                nc.tensor.matmul(pb, aT_0[:, s_sl], wp0_sb[:, 512:768],
                                 start=True, stop=True)
                ost_idx = 0
            else:
                nc.tensor.matmul(pa, aT_bc[:, s_sl], wp12_sb[:, 0:512],
                                 start=True, stop=True)
                nc.tensor.matmul(pb, aT_bc[:, s_sl], wp12_sb[:, 512:768],
                                 start=True, stop=True)
                ost_idx = 1
            if sc % 4 == 0:
                ostage_cur[ost_idx] = opool.tile([P, 4, NX], BF16, tag="o",
                                                 name=f"os_{hh}_{sc}")
            ostage = ostage_cur[ost_idx]
            nc.vector.tensor_copy(ostage[:, sc % 4, 0:512], pa)
            if use_act:
                nc.scalar.copy(ostage[:, sc % 4, 512:768], pb)
            else:
                nc.vector.tensor_copy(ostage[:, sc % 4, 512:768], pb)
            if sc % 4 == 3:
                orows = outs_d[ost_idx][(sc - 3) * 128:(sc + 1) * 128, :]
                nc.sync.dma_start(
                    orows.rearrange("(c p) n -> p c n", p=P), ostage[:])

        # ---- emission order = per-engine FIFO order = pipeline order ----
        # One long exp stream (24 t-chunk pairs across 3 heads). Per step k:
        # the NEXT pair's score tiles, 1:1-interleaved with filler units
        # (v_proj / later-head qk_proj, each a ps_score alloc that trickles
        # with the exp-paced pool rotation), then PV for pair k-2 (its exps
        # long done -> a dependency-free dense burst that keeps the PE warm).
        # Head-2 PV runs a catch-up schedule so the last pair is JIT and the
        # tail starts right after the last exp. Norms are deferred two pairs
        # into the next window and MUST be emitted before the next head's
        # accumulators are allocated (pool-slot reuse).
        for qc in range(QC):
            qk_proj(0, [qc], dup_now=True)
            scores_exp_pair(0, 0, qqs=[qc])

        def vu(t):
            return lambda: v_proj([t])

        def qku(mc, qc):
            return lambda: qk_proj(mc, [qc], dup_now=True)

        def pu(hh, sc):
            return lambda: proj_head(hh, sc)

        unit_sched = {
            0: [vu(0), vu(1), vu(2), vu(3)],
            1: [vu(4), vu(5), vu(6), vu(7)],
            2: [vu(8), vu(9), vu(10)],
            3: [vu(11), vu(12), vu(13)],
            4: [vu(14), vu(15), qku(1, 0)],
            5: [qku(1, 1), qku(1, 2)],
            6: [qku(1, 3)],
            8: [qku(2, 0), qku(2, 1)],
            9: [qku(2, 2), qku(2, 3)],
        }
        # head-0 projection during w1/w2 (needs only norm(0), done at k=11)
        for k in range(12, 20):
            unit_sched[k] = [pu(0, 2 * (k - 12)), pu(0, 2 * (k - 12) + 1)]
        pv_sched = {k: [k - 2] for k in range(2, 18)}
        pv_sched.update({18: [16, 17], 19: [18], 20: [19, 20],
                         21: [21], 22: [22], 23: [23]})
        norm_a_sched = {9: 0, 17: 1}   # after PV pair 8h+7 lands (lag-2)
        norm_b_sched = {11: 0, 19: 1}  # two steps later: recip already done
        NK = HPC * NPAIR
        pvs_store = {}
        norm_st = {}

        for k in range(NK):
            us = list(unit_sched.get(k, []))
            if k + 1 < NK:
                h2, j2 = divmod(k + 1, NPAIR)
                for qq in range(QC):
                    scores_exp_pair(h2, j2, qqs=[qq])
                    if us:
                        us.pop(0)()
            for u in us:
                u()
            if k in norm_b_sched:
                hn = norm_b_sched[k]
                norm_stage_b(hn, *norm_st.pop(hn))
            for p in pv_sched.get(k, []):
                ph, pj = divmod(p, NPAIR)
                if pj == 0:
                    pvs_store[ph] = [ps_pv.tile([P, 512], F32, tag="pv",
                                                name=f"pv_{ph}_{qc}")
                                     for qc in range(QC)]
                    warm_state["pvs"] = pvs_store[ph]
                pv_chunk(ph, 2 * pj, pvs_store[ph])
                pv_chunk(ph, 2 * pj + 1, pvs_store[ph])
            if k in norm_a_sched:
                hn = norm_a_sched[k]
                norm_st[hn] = norm_stage_a(hn, pvs_store.pop(hn))

        # ---- tail: norm(2) + head-2 output projection ----
        pvs2 = pvs_store.pop(2)
        rr2, pvsbs2 = norm_stage_a(2, pvs2)
        # warmkeeper matmuls staggered on the staging copies / reciprocal
        # halves so the PE keeps HAM busy through the ~7us norm window
        for i, dep in enumerate(pvsbs2 + [rr2, rr2]):
            dmy = ps_pv.tile([P, 512], F32, tag="pv", name=f"dmy_{i}")
            if i < QC:
                src = dep[0:1, 0:512]
            elif i == QC:
                src = dep[0:1, 0:256]
            else:
                src = dep[0:1, 256:512]
            nc.tensor.matmul(dmy[0:1, 0:256 if i >= QC else 512],
                             src[:, 0:1], src, start=True, stop=True)
        norm_stage_b(2, rr2, pvsbs2, pool=ps_pv)
        for sc in range(S // 128):
            proj_head(2, sc, use_act=True)


# ---------------------------------------------------------------------------
# host side
# ---------------------------------------------------------------------------

def make_in_maps(hidden_states, w_attn, b_attn, w_proj, S=2048,
                 use_bias=False):
    """Build the 8 per-core input dicts (numpy bf16)."""
    bf = ml_dtypes.bfloat16
    KCH = 7 if use_bias else 6
    KDIM = KCH * 128
    hidden = np.asarray(hidden_states)
    w_attn = np.asarray(w_attn)
    b_attn = np.asarray(b_attn)
    w_proj = np.asarray(w_proj)

    xts = []
    for b in range(hidden.shape[0]):
        xt = np.zeros((KDIM, S), dtype=bf)
        xt[0:NX, :] = hidden[b].T.astype(bf)
        if use_bias:
            xt[NX, :] = 1.0
        # partition-major layout: [p, qc, c, tok]
        xt = np.ascontiguousarray(
            xt.reshape(KCH, 128, S // 512, 512).transpose(1, 2, 0, 3))
        xts.append(xt)

    in_maps = []
    for c in range(N_CORES):
        b = c // (N_CORES // hidden.shape[0])
        h0 = HPC * (c % (N_CORES // hidden.shape[0]))
        wqk = np.zeros((KDIM, 6 * D), dtype=bf)
        wv = np.zeros((KDIM, HPC * D), dtype=bf)
        for i in range(HPC):
            h = h0 + i
            wqk[0:NX, (2 * i) * D:(2 * i + 1) * D] = \
                w_attn[:, h * D:(h + 1) * D].astype(bf)
            wqk[0:NX, (2 * i + 1) * D:(2 * i + 2) * D] = \
                w_attn[:, NX + h * D:NX + (h + 1) * D].astype(bf)
            wv[0:NX, i * D:(i + 1) * D] = \
                w_attn[:, 2 * NX + h * D:2 * NX + (h + 1) * D].astype(bf)
            if use_bias:
                wqk[NX, (2 * i) * D:(2 * i + 1) * D] = \
                    b_attn[h * D:(h + 1) * D].astype(bf)
                wqk[NX, (2 * i + 1) * D:(2 * i + 2) * D] = \
                    b_attn[NX + h * D:NX + (h + 1) * D].astype(bf)
                wv[NX, i * D:(i + 1) * D] = \
                    b_attn[2 * NX + h * D:2 * NX + (h + 1) * D].astype(bf)
        wp = w_proj[h0 * D:(h0 + HPC) * D, :].astype(bf)
        in_maps.append({"xt": xts[b], "wqk": wqk, "wv": wv, "wp": wp})
    return in_maps


_CACHE = {}


def kernel(hidden_states, w_attn, b_attn, w_proj, b_proj):
    from concourse.bass_utils import run_bass_kernel_spmd

    hidden = np.asarray(hidden_states, dtype=np.float32)
    B, S, _ = hidden.shape
    use_bias = bool(np.any(np.asarray(b_attn)))
    in_maps = make_in_maps(hidden, w_attn, b_attn, w_proj, S=S,
                           use_bias=use_bias)

    key = (S, use_bias)
    if key not in _CACHE:
        _CACHE[key] = build_nc(S=S, use_bias=use_bias)
    nc = _CACHE[key]

    res = run_bass_kernel_spmd(nc, in_maps, core_ids=list(range(N_CORES)))
    out = gather_out(res.results, B, S)
    out += np.asarray(b_proj, dtype=np.float32)
    return out


def gather_out(results, B, S):
    cpb = N_CORES // B
    out = np.zeros((B, S, NX), dtype=np.float32)
    for c in range(N_CORES):
        for h in range(2):
            out[c // cpb] += np.asarray(results[c][f"out{h}"],
                                        dtype=np.float32)
    return out


# revision 58
# speedup vs baseline: 1.2148x; 1.2148x over previous
"""Multi-head attention (B=2, S=2048, nx=768, H=12) on 8 TRN2 NeuronCores.

Sharding: 24 (batch, head) pairs -> 3 heads per core. Core c handles batch
c//4, heads {3*(c%4), +1, +2}. Each core computes QKV projection for its
head slice, attention, and a partial output projection (its 192 rows of
w_proj); the host sums the 4 partials per batch and adds b_proj.

v2 schedule (ACT-exp is the ~107us floor; everything else overlaps it):
  - chunked priority DMA: wqk first, then xt by 512-token chunk, so
    qk_proj(0) starts ~2us in and the first exp lands ~8us in.
  - scores psum tiles are [128, 1024] = (even t-chunk 512q | odd 512q);
    ONE exp per tile means both matmuls of the next tile wait on the same
    semaphore and run CONCURRENTLY as PE row-tiles (rows 0:64 even chunks,
    64:128 odd) - measured 132 ns/MM vs 261 serial.
  - v_proj + qk_proj(1,2) are emitted as PE filler inside the head-0
    scores window; the per-head loop interleaves PV(h) with scores(h+1).
  - KCH=6: the zero-bias fast path drops the bias/pad contraction chunk
    (b_attn is zeros per spec); a KCH=7 bias build is compiled on demand.
  - tail: proj K=128 (heads 0,1) matmuls pre-issued during head-2's PV,
    norm staging for head 2 on ACT (idle post-exp), the 4 reciprocal-
    broadcast matmuls issued back-to-back at row positions 0/32/64/96
    (concurrent), output staged to bf16 and summed on host in f32.
"""

import numpy as np
import ml_dtypes

import concourse.bass as bass
import concourse.tile as tile
import concourse.mybir as mybir
from concourse import bacc

BF16 = mybir.dt.bfloat16
F32 = mybir.dt.float32

NX = 768
D = 64
HPC = 3          # heads per core
N_CORES = 8


def build_nc(S=2048, use_bias=False):
    """Build the single-core SPMD program."""
    KCH = 7 if use_bias else 6   # contraction chunks of 128
    KDIM = KCH * 128
    nc = bacc.Bacc("TRN2", target_bir_lowering=False, debug=False)

    # xt is partition-major on the host: [p, qc, c, tok] so each per-qc DMA
    # reads 6KB-contiguous per partition (descriptor-efficient).
    xt_d = nc.dram_tensor("xt", [128, S // 512, KCH, 512], BF16,
                          kind="ExternalInput")
    wqk_d = nc.dram_tensor("wqk", [KDIM, 6 * D], BF16, kind="ExternalInput")
    wv_d = nc.dram_tensor("wv", [KDIM, HPC * D], BF16, kind="ExternalInput")
    wp_d = nc.dram_tensor("wp", [HPC * D, NX], BF16, kind="ExternalInput")
    # two partial outputs: head-0's projection (runs during windows 1-2)
    # and heads 1+2 (tail); the host sums them
    outs_d = [nc.dram_tensor(f"out{h}", [S, NX], BF16, kind="ExternalOutput")
              for h in range(2)]

    with tile.TileContext(nc) as tc:
        _build_body(tc, [o.ap() for o in outs_d], xt_d.ap(), wqk_d.ap(),
                    wv_d.ap(), wp_d.ap(), S, KCH)
    nc.compile()
    return nc


def _build_body(tc, outs_d, xt_d, wqk_d, wv_d, wp_d, S, KCH):
    nc = tc.nc
    P = 128
    TC = S // 128    # t (key) chunks
    QC = S // 512    # q chunks of 512
    NPAIR = TC // 2  # t-chunk pairs per head

    with tc.tile_pool(name="const", bufs=1) as cpool, \
         tc.tile_pool(name="epool", bufs=NPAIR + 2) as epool, \
         tc.tile_pool(name="small", bufs=4) as spool, \
         tc.tile_pool(name="ostage", bufs=3) as opool, \
         tc.tile_pool(name="pvpool", bufs=QC + 1) as pvpool, \
         tc.tile_pool(name="ps_score", bufs=2, space="PSUM") as ps_score, \
         tc.tile_pool(name="ps_pv", bufs=QC, space="PSUM") as ps_pv:

        # ---- stage inputs in SBUF (priority order: wqk, xt chunks, ...) ----
        wqk_sb = cpool.tile([P, KCH, 6 * D], BF16)
        wqk_r = wqk_d.rearrange("(c p) m -> p c m", p=P)
        nc.sync.dma_start(wqk_sb[:, :, 0:128], wqk_r[:, :, 0:128])
        xt_sb = cpool.tile([P, KCH, S], BF16)
        for qc in range(QC):
            sl = slice(qc * 512, (qc + 1) * 512)
            nc.sync.dma_start(xt_sb[:, :, sl], xt_d[:, qc, :, :])
        nc.sync.dma_start(wqk_sb[:, :, 128:384], wqk_r[:, :, 128:384])
        wv_sb = cpool.tile([P, KCH, HPC * D], BF16)
        nc.sync.dma_start(wv_sb[:], wv_d.rearrange("(c p) m -> p c m", p=P))
        wp0_sb = cpool.tile([D, NX], BF16)
        nc.sync.dma_start(wp0_sb[:], wp_d[0:D, :])
        wp12_sb = cpool.tile([P, NX], BF16)
        nc.sync.dma_start(wp12_sb[:], wp_d[D:HPC * D, :])
        ones4 = cpool.tile([97, D], F32)
        nc.vector.memset(ones4[:], 1.0)

        # q2: Q^T duplicated into both partition halves (rows 0:64 == 64:128)
        # k2: K^T with even token-chunks in rows 0:64, odd in rows 64:128 --
        # the stationary layout for the row-paired scores matmuls.
        q2_sb = cpool.tile([P, HPC, S], BF16)
        k2_sb = cpool.tile([P, HPC, S // 2], BF16)
        v_sb = cpool.tile([P, TC, HPC, D + 1], BF16)
        aT_0 = cpool.tile([D, S], BF16)    # head 0 (projected during w1-w2)
        aT_bc = cpool.tile([P, S], BF16)   # heads 1,2 stacked (tail proj)

        # wqk col order is [qA kA qB kB qC kC]; m-chunk mc covers head mc's
        # q (psum partitions 0:64) and k (64:128). Transient psums all come
        # from ps_score so ps_pv holds only the long-lived PV accumulators.
        def qk_proj(mc, qcs, dup_now=False):
            for qc in qcs:
                ps = ps_score.tile([P, 1024], F32, tag="score",
                                   name=f"qk_{mc}_{qc}")
                for kc in range(KCH):
                    nc.tensor.matmul(
                        ps[:, 0:512],
                        wqk_sb[:, kc, mc * 128:(mc + 1) * 128],
                        xt_sb[:, kc, qc * 512:(qc + 1) * 512],
                        start=(kc == 0), stop=(kc == KCH - 1))
                nc.vector.tensor_copy(q2_sb[0:D, mc, qc * 512:(qc + 1) * 512],
                                      ps[0:D, 0:512])
                kview = ps[D:P, 0:512].rearrange("p (b c) -> p b c", c=128)
                k2w = k2_sb[:, mc, qc * 256:(qc + 1) * 256].rearrange(
                    "p (b c) -> p b c", c=128)
                nc.vector.tensor_copy(k2w[0:D], kview[:, 0::2, :])
                nc.vector.tensor_copy(k2w[D:P], kview[:, 1::2, :])
                if dup_now:
                    nc.vector.tensor_copy(
                        q2_sb[D:P, mc, qc * 512:(qc + 1) * 512],
                        q2_sb[0:D, mc, qc * 512:(qc + 1) * 512])
            if not dup_now and qcs[-1] == QC - 1:
                # duplicate q into the lower half (DVE 4x bf16 copy)
                nc.vector.tensor_copy(q2_sb[D:P, mc, :], q2_sb[0:D, mc, :])

        def v_proj(ts):
            if ts[0] == 0:
                nc.vector.memset(v_sb[:, :, :, D:D + 1], 1.0)
            for t in ts:
                ps = ps_score.tile([P, 1024], F32, tag="score", name=f"v_{t}")
                for kc in range(KCH):
                    nc.tensor.matmul(
                        ps[:, 0:HPC * D],
                        xt_sb[:, kc, t * 128:(t + 1) * 128],
                        wv_sb[:, kc, :],
                        start=(kc == 0), stop=(kc == KCH - 1))
                nc.vector.tensor_copy(
                    v_sb[:, t, :, 0:D],
                    ps[:, 0:HPC * D].rearrange("p (h d) -> p h d", h=HPC))

        e_tiles = {}
        warm_state = {"pvs": None, "i": 0}

        def warm_pulse(e2, qq):
            # HAM warmkeeper: a K=1 N=512 matmul (~213ns) gated on an exp
            # of the NEXT pair -- fires at the same semaphore points the
            # following step's score matmuls wait on, filling the PE's
            # trailing idle without delaying any real work.
            pvs = warm_state["pvs"]
            if pvs is None:
                return
            i = warm_state["i"] = (warm_state["i"] + 1) % QC
            nc.tensor.matmul(pvs[i][96:97, :],
                             e2[0:1, qq, 0, 0:1], e2[0:1, qq, 0, :],
                             start=True, stop=True, skip_group_check=True,
                             tile_position=(0, 96))

        def scores_exp_pair(h, j, qqs=None):
            # t-chunks (2j, 2j+1) as concurrent PE row-tiles: even chunk from
            # k2 rows 0:64 -> psum cols 0:512, odd from rows 64:128 -> cols
            # 512:1024. One exp per psum tile keeps the pair on one sem, so
            # the two matmuls issue back-to-back and overlap in the array.
            # e2 dims [qq, parity, tok] keep the exp output AP contiguous.
            if qqs is None:
                qqs = range(QC)
            if (h, 2 * j) in e_tiles:
                e2 = e_tiles[(h, 2 * j)]
            else:
                e2 = epool.tile([P, QC, 2, 512], BF16, tag="E",
                                name=f"e_{h}_{j}")
                e_tiles[(h, 2 * j)] = e2
            for qq in qqs:
                ps = ps_score.tile([P, 1024], F32, tag="score", name="sc")
                qsl = slice(qq * 512, (qq + 1) * 512)
                nc.tensor.matmul(
                    ps[:, 0:512],
                    k2_sb[0:D, h, j * 128:(j + 1) * 128],
                    q2_sb[0:D, h, qsl], start=True, stop=True)
                nc.tensor.matmul(
                    ps[:, 512:1024],
                    k2_sb[D:P, h, j * 128:(j + 1) * 128],
                    q2_sb[D:P, h, qsl], start=True, stop=True)
                nc.scalar.activation(
                    e2[:, qq, :, :], ps[:],
                    mybir.ActivationFunctionType.Exp, scale=0.125)

        def pv_chunk(h, t, pvs):
            e2 = e_tiles[(h, t & ~1)]
            for qc in range(QC):
                nc.tensor.matmul(
                    pvs[qc][0:D + 1, :],
                    v_sb[:, t, h, :],
                    e2[:, qc, t & 1, :],
                    start=(t == 0), stop=(t == TC - 1))
            if t & 1:
                e_tiles.pop((h, t & ~1))

        # sumexp rows gathered at 32-aligned partitions -> one batched
        # reciprocal; 1/Z broadcast via K=1 matmuls at row positions
        # 0/32/64/96. Stage a (copies + reciprocal) releases the PV
        # accumulators; stage b (rb matmuls + mults) is emitted two pairs
        # later so the PE never FIFO-blocks on the reciprocal. Head 2's
        # staging runs on both ACT (idle after the last exp) and DVE.
        def norm_stage_a(h, pvs):
            if h == HPC - 1:
                def stage_copy(dst, src, qc):
                    if qc < 2:
                        nc.vector.tensor_copy(dst, src)
                    else:
                        nc.scalar.copy(dst, src)
            else:
                def stage_copy(dst, src, qc):
                    nc.vector.tensor_copy(dst, src)
            rt = spool.tile([97, 512], F32, tag="rt")
            nc.vector.memset(rt[:], 1.0)
            pvsbs = []
            for qc in range(QC):
                pvsb = pvpool.tile([D, 512], F32, tag="pvsb",
                                   name=f"pvsb_{h}_{qc}")
                stage_copy(pvsb[:], pvs[qc][0:D, :], qc)
                stage_copy(rt[32 * qc:32 * qc + 1, :],
                           pvs[qc][D:D + 1, :], qc)
                pvsbs.append(pvsb)
            rr = spool.tile([97, 512], F32, tag="rr")
            if h == HPC - 1:
                # split so warmkeeper matmuls can stagger through it
                nc.vector.reciprocal(rr[:, 0:256], rt[:, 0:256])
                nc.vector.reciprocal(rr[:, 256:512], rt[:, 256:512])
            else:
                nc.vector.reciprocal(rr[:], rt[:])
            return rr, pvsbs

        def norm_stage_b(h, rr, pvsbs, pool=None):
            for qc in range(QC):
                if pool is None:
                    rb = ps_score.tile([P, 1024], F32, tag="score",
                                       name=f"rb_{h}_{qc}")
                else:
                    rb = pool.tile([P, 512], F32, tag="pv",
                                   name=f"rb_{h}_{qc}")
                nc.tensor.matmul(rb[0:D, 0:512],
                                 ones4[32 * qc:32 * qc + 1, :],
                                 rr[32 * qc:32 * qc + 1, :],
                                 start=True, stop=True,
                                 tile_position=(32 * qc, 0))
                qsl = slice(qc * 512, (qc + 1) * 512)
                dst = (aT_0[:, qsl] if h == 0
                       else aT_bc[(h - 1) * D:h * D, qsl])
                nc.vector.tensor_tensor(dst, pvsbs[qc][:], rb[0:D, 0:512],
                                        mybir.AluOpType.mult)

        # Per-head output projection: head hh's 64 aT rows x its 64 rows of
        # w_proj (K=64 matmuls). Head 0 runs during window 1, head 1 during
        # late window 2 (both only need their own norm), head 2 in the tail.
        # Staging converts to bf16; 4 chunks batch into one DMA issue.
        ostage_cur = {}

        def proj_head(hh, sc, use_act=False):
            # hh=0: head-0 rows only (runs during w1, stream out0)
            # hh=2: heads 1+2 accumulated in one psum (tail, stream out1)
            s_sl = slice(sc * 128, (sc + 1) * 128)
            if hh == 2 and sc % 4 >= 2:
                ta = ps_pv.tile([P, 512], F32, tag="pv", name=f"pja_{sc}")
                tb = ps_pv.tile([P, 512], F32, tag="pv", name=f"pjb_{sc}")
                pa, pb = ta[:], tb[:, 0:256]
            else:
                ps = ps_score.tile([P, 1024], F32, tag="score",
                                   name=f"pj_{hh}_{sc}")
                pa, pb = ps[:, 0:512], ps[:, 512:768]
            if hh == 0:
                nc.tensor.matmul(pa, aT_0[:, s_sl], wp0_sb[:, 0:512],
                                 start=True, stop=True)
                nc.tensor.matmul(pb, aT_0[:, s_sl], wp0_sb[:, 512:768],
                                 start=True, stop=True)
                ost_idx = 0
            else:
                nc.tensor.matmul(pa, aT_bc[:, s_sl], wp12_sb[:, 0:512],
                                 start=True, stop=True)
                nc.tensor.matmul(pb, aT_bc[:, s_sl], wp12_sb[:, 512:768],
                                 start=True, stop=True)
                ost_idx = 1
            if sc % 4 == 0:
                ostage_cur[ost_idx] = opool.tile([P, 4, NX], BF16, tag="o",
                                                 name=f"os_{hh}_{sc}")
            ostage = ostage_cur[ost_idx]
            nc.vector.tensor_copy(ostage[:, sc % 4, 0:512], pa)
            if use_act:
                nc.scalar.copy(ostage[:, sc % 4, 512:768], pb)
            else:
                nc.vector.tensor_copy(ostage[:, sc % 4, 512:768], pb)
            if sc % 4 == 3:
                orows = outs_d[ost_idx][(sc - 3) * 128:(sc + 1) * 128, :]
                nc.sync.dma_start(
                    orows.rearrange("(c p) n -> p c n", p=P), ostage[:])

        # ---- emission order = per-engine FIFO order = pipeline order ----
        # One long exp stream (24 t-chunk pairs across 3 heads). Per step k:
        # the NEXT pair's score tiles, 1:1-interleaved with filler units
        # (v_proj / later-head qk_proj, each a ps_score alloc that trickles
        # with the exp-paced pool rotation), then PV for pair k-2 (its exps
        # long done -> a dependency-free dense burst that keeps the PE warm).
        # Head-2 PV runs a catch-up schedule so the last pair is JIT and the
        # tail starts right after the last exp. Norms are deferred two pairs
        # into the next window and MUST be emitted before the next head's
        # accumulators are allocated (pool-slot reuse).
        for qc in range(QC):
            qk_proj(0, [qc], dup_now=True)
            scores_exp_pair(0, 0, qqs=[qc])

        def vu(t):
            return lambda: v_proj([t])

        def qku(mc, qc):
            return lambda: qk_proj(mc, [qc], dup_now=True)

        def pu(hh, sc):
            return lambda: proj_head(hh, sc)

        unit_sched = {
            0: [vu(0), vu(1), vu(2), vu(3)],
            1: [vu(4), vu(5), vu(6), vu(7)],
            2: [vu(8), vu(9), vu(10)],
            3: [vu(11), vu(12), vu(13)],
            4: [vu(14), vu(15), qku(1, 0)],
            5: [qku(1, 1), qku(1, 2)],
            6: [qku(1, 3)],
            8: [qku(2, 0), qku(2, 1)],
            9: [qku(2, 2), qku(2, 3)],
        }
        # head-0 projection during w1/w2 (needs only norm(0), done at k=11)
        for k in range(12, 20):
            unit_sched[k] = [pu(0, 2 * (k - 12)), pu(0, 2 * (k - 12) + 1)]
        pv_sched = {k: [k - 2] for k in range(2, 18)}
        pv_sched.update({18: [16, 17], 19: [18], 20: [19, 20],
                         21: [21], 22: [22], 23: [23]})
        norm_a_sched = {9: 0, 17: 1}   # after PV pair 8h+7 lands (lag-2)
        norm_b_sched = {11: 0, 19: 1}  # two steps later: recip already done
        NK = HPC * NPAIR
        pvs_store = {}
        norm_st = {}

        for k in range(NK):
            us = list(unit_sched.get(k, []))
            if k + 1 < NK:
                h2, j2 = divmod(k + 1, NPAIR)
                for qq in range(QC):
                    scores_exp_pair(h2, j2, qqs=[qq])
                    if us:
                        us.pop(0)()
            for u in us:
                u()
            if k in norm_b_sched:
                hn = norm_b_sched[k]
                norm_stage_b(hn, *norm_st.pop(hn))
            for p in pv_sched.get(k, []):
                ph, pj = divmod(p, NPAIR)
                if pj == 0:
                    pvs_store[ph] = [ps_pv.tile([P, 512], F32, tag="pv",
                                                name=f"pv_{ph}_{qc}")
                                     for qc in range(QC)]
                    warm_state["pvs"] = pvs_store[ph]
                pv_chunk(ph, 2 * pj, pvs_store[ph])
                pv_chunk(ph, 2 * pj + 1, pvs_store[ph])
            if k in norm_a_sched:
                hn = norm_a_sched[k]
                norm_st[hn] = norm_stage_a(hn, pvs_store.pop(hn))
            if k + 1 < NK:
                h2, j2 = divmod(k + 1, NPAIR)
                e2n = e_tiles.get((h2, 2 * j2))
                if e2n is not None:
                    warm_pulse(e2n, 0)
                    warm_pulse(e2n, 1)

        # ---- tail: norm(2) + head-2 output projection ----
        pvs2 = pvs_store.pop(2)
        rr2, pvsbs2 = norm_stage_a(2, pvs2)
        # warmkeeper matmuls staggered on the staging copies / reciprocal
        # halves so the PE keeps HAM busy through the ~7us norm window
        for i, dep in enumerate(pvsbs2 + [rr2, rr2]):
            dmy = ps_pv.tile([P, 512], F32, tag="pv", name=f"dmy_{i}")
            if i < QC:
                src = dep[0:1, 0:512]
            elif i == QC:
                src = dep[0:1, 0:256]
            else:
                src = dep[0:1, 256:512]
            nc.tensor.matmul(dmy[0:1, 0:256 if i >= QC else 512],
                             src[:, 0:1], src, start=True, stop=True)
        norm_stage_b(2, rr2, pvsbs2, pool=ps_pv)
        for sc in range(S // 128):
            proj_head(2, sc, use_act=True)


# ---------------------------------------------------------------------------
# host side
# ---------------------------------------------------------------------------

def make_in_maps(hidden_states, w_attn, b_attn, w_proj, S=2048,
                 use_bias=False):
    """Build the 8 per-core input dicts (numpy bf16)."""
    bf = ml_dtypes.bfloat16
    KCH = 7 if use_bias else 6
    KDIM = KCH * 128
    hidden = np.asarray(hidden_states)
    w_attn = np.asarray(w_attn)
    b_attn = np.asarray(b_attn)
    w_proj = np.asarray(w_proj)

    xts = []
    for b in range(hidden.shape[0]):
        xt = np.zeros((KDIM, S), dtype=bf)
        xt[0:NX, :] = hidden[b].T.astype(bf)
        if use_bias:
            xt[NX, :] = 1.0
        # partition-major layout: [p, qc, c, tok]
        xt = np.ascontiguousarray(
            xt.reshape(KCH, 128, S // 512, 512).transpose(1, 2, 0, 3))
        xts.append(xt)

    in_maps = []
    for c in range(N_CORES):
        b = c // (N_CORES // hidden.shape[0])
        h0 = HPC * (c % (N_CORES // hidden.shape[0]))
        wqk = np.zeros((KDIM, 6 * D), dtype=bf)
        wv = np.zeros((KDIM, HPC * D), dtype=bf)
        for i in range(HPC):
            h = h0 + i
            wqk[0:NX, (2 * i) * D:(2 * i + 1) * D] = \
                w_attn[:, h * D:(h + 1) * D].astype(bf)
            wqk[0:NX, (2 * i + 1) * D:(2 * i + 2) * D] = \
                w_attn[:, NX + h * D:NX + (h + 1) * D].astype(bf)
            wv[0:NX, i * D:(i + 1) * D] = \
                w_attn[:, 2 * NX + h * D:2 * NX + (h + 1) * D].astype(bf)
            if use_bias:
                wqk[NX, (2 * i) * D:(2 * i + 1) * D] = \
                    b_attn[h * D:(h + 1) * D].astype(bf)
                wqk[NX, (2 * i + 1) * D:(2 * i + 2) * D] = \
                    b_attn[NX + h * D:NX + (h + 1) * D].astype(bf)
                wv[NX, i * D:(i + 1) * D] = \
                    b_attn[2 * NX + h * D:2 * NX + (h + 1) * D].astype(bf)
        wp = w_proj[h0 * D:(h0 + HPC) * D, :].astype(bf)
        in_maps.append({"xt": xts[b], "wqk": wqk, "wv": wv, "wp": wp})
    return in_maps


_CACHE = {}


def kernel(hidden_states, w_attn, b_attn, w_proj, b_proj):
    from concourse.bass_utils import run_bass_kernel_spmd

    hidden = np.asarray(hidden_states, dtype=np.float32)
    B, S, _ = hidden.shape
    use_bias = bool(np.any(np.asarray(b_attn)))
    in_maps = make_in_maps(hidden, w_attn, b_attn, w_proj, S=S,
                           use_bias=use_bias)

    key = (S, use_bias)
    if key not in _CACHE:
        _CACHE[key] = build_nc(S=S, use_bias=use_bias)
    nc = _CACHE[key]

    res = run_bass_kernel_spmd(nc, in_maps, core_ids=list(range(N_CORES)))
    out = gather_out(res.results, B, S)
    out += np.asarray(b_proj, dtype=np.float32)
    return out


def gather_out(results, B, S):
    cpb = N_CORES // B
    out = np.zeros((B, S, NX), dtype=np.float32)
    for c in range(N_CORES):
        for h in range(2):
            out[c // cpb] += np.asarray(results[c][f"out{h}"],
                                        dtype=np.float32)
    return out


# revision 70
# speedup vs baseline: 1.3720x; 1.1294x over previous
"""Multi-head attention (B=2, S=2048, nx=768, H=12) on 8 TRN2 NeuronCores.

Sharding: 24 (batch, head) pairs -> 3 heads per core. Core c handles batch
c//4, heads {3*(c%4), +1, +2}. Each core computes QKV projection for its
head slice, attention, and a partial output projection (its 192 rows of
w_proj); the host sums the 4 partials per batch and adds b_proj.

v2 schedule (ACT-exp is the ~107us floor; everything else overlaps it):
  - chunked priority DMA: wqk first, then xt by 512-token chunk, so
    qk_proj(0) starts ~2us in and the first exp lands ~8us in.
  - scores psum tiles are [128, 1024] = (even t-chunk 512q | odd 512q);
    ONE exp per tile means both matmuls of the next tile wait on the same
    semaphore and run CONCURRENTLY as PE row-tiles (rows 0:64 even chunks,
    64:128 odd) - measured 132 ns/MM vs 261 serial.
  - v_proj + qk_proj(1,2) are emitted as PE filler inside the head-0
    scores window; the per-head loop interleaves PV(h) with scores(h+1).
  - KCH=6: the zero-bias fast path drops the bias/pad contraction chunk
    (b_attn is zeros per spec); a KCH=7 bias build is compiled on demand.
  - tail: proj K=128 (heads 0,1) matmuls pre-issued during head-2's PV,
    norm staging for head 2 on ACT (idle post-exp), the 4 reciprocal-
    broadcast matmuls issued back-to-back at row positions 0/32/64/96
    (concurrent), output staged to bf16 and summed on host in f32.
"""

import numpy as np
import ml_dtypes

import concourse.bass as bass
import concourse.tile as tile
import concourse.mybir as mybir
from concourse import bacc

BF16 = mybir.dt.bfloat16
F32 = mybir.dt.float32

NX = 768
D = 64
HPC = 3          # heads per core
N_CORES = 8


def build_nc(S=2048, use_bias=False, esc=0.125):
    """Build the single-core SPMD program."""
    KCH = 7 if use_bias else 6   # contraction chunks of 128
    KDIM = KCH * 128
    nc = bacc.Bacc("TRN2", target_bir_lowering=False, debug=False)

    # xt is partition-major on the host: [p, qc, c, tok] so each per-qc DMA
    # reads 6KB-contiguous per partition (descriptor-efficient).
    xt_d = nc.dram_tensor("xt", [128, S // 512, KCH, 512], BF16,
                          kind="ExternalInput")
    wqk_d = nc.dram_tensor("wqk", [KDIM, 6 * D], BF16, kind="ExternalInput")
    wv_d = nc.dram_tensor("wv", [KDIM, HPC * D], BF16, kind="ExternalInput")
    wp_d = nc.dram_tensor("wp", [HPC * D, NX], BF16, kind="ExternalInput")
    # two partial outputs: head-0's projection (runs during windows 1-2)
    # and heads 1+2 (tail); the host sums them
    outs_d = [nc.dram_tensor(f"out{h}", [S, NX], BF16, kind="ExternalOutput")
              for h in range(2)]

    with tile.TileContext(nc) as tc:
        _build_body(tc, [o.ap() for o in outs_d], xt_d.ap(), wqk_d.ap(),
                    wv_d.ap(), wp_d.ap(), S, KCH, esc)
    nc.compile()
    return nc


def _build_body(tc, outs_d, xt_d, wqk_d, wv_d, wp_d, S, KCH, esc):
    nc = tc.nc
    P = 128
    TC = S // 128    # t (key) chunks
    QC = S // 512    # q chunks of 512
    NPAIR = TC // 2  # t-chunk pairs per head

    with tc.tile_pool(name="const", bufs=1) as cpool, \
         tc.tile_pool(name="epool", bufs=NPAIR + 2) as epool, \
         tc.tile_pool(name="small", bufs=4) as spool, \
         tc.tile_pool(name="ostage", bufs=3) as opool, \
         tc.tile_pool(name="pvpool", bufs=QC + 1) as pvpool, \
         tc.tile_pool(name="ps_score", bufs=2, space="PSUM") as ps_score, \
         tc.tile_pool(name="ps_pv", bufs=QC, space="PSUM") as ps_pv:

        # ---- stage inputs in SBUF (priority order: wqk, xt chunks, ...) ----
        wqk_sb = cpool.tile([P, KCH, 6 * D], BF16)
        wqk_r = wqk_d.rearrange("(c p) m -> p c m", p=P)
        nc.sync.dma_start(wqk_sb[:, :, 0:128], wqk_r[:, :, 0:128])
        xt_sb = cpool.tile([P, KCH, S], BF16)
        for qc in range(QC):
            sl = slice(qc * 512, (qc + 1) * 512)
            nc.sync.dma_start(xt_sb[:, :, sl], xt_d[:, qc, :, :])
        nc.sync.dma_start(wqk_sb[:, :, 128:384], wqk_r[:, :, 128:384])
        wv_sb = cpool.tile([P, KCH, HPC * D], BF16)
        nc.sync.dma_start(wv_sb[:], wv_d.rearrange("(c p) m -> p c m", p=P))
        wp0_sb = cpool.tile([D, NX], BF16)
        nc.sync.dma_start(wp0_sb[:], wp_d[0:D, :])
        wp12_sb = cpool.tile([P, NX], BF16)
        nc.sync.dma_start(wp12_sb[:], wp_d[D:HPC * D, :])
        ones4 = cpool.tile([97, D], F32)
        nc.vector.memset(ones4[:], 1.0)

        # q2: Q^T duplicated into both partition halves (rows 0:64 == 64:128)
        # k2: K^T with even token-chunks in rows 0:64, odd in rows 64:128 --
        # the stationary layout for the row-paired scores matmuls.
        q2_sb = cpool.tile([P, HPC, S], BF16)
        k2_sb = cpool.tile([P, HPC, S // 2], BF16)
        v_sb = cpool.tile([P, TC, HPC, D + 1], BF16)
        aT_0 = cpool.tile([D, S], BF16)    # head 0 (projected during w1-w2)
        aT_bc = cpool.tile([P, S], BF16)   # heads 1,2 stacked (tail proj)

        # wqk col order is [qA kA qB kB qC kC]; m-chunk mc covers head mc's
        # q (psum partitions 0:64) and k (64:128). Transient psums all come
        # from ps_score so ps_pv holds only the long-lived PV accumulators.
        def qk_proj(mc, qcs, dup_now=False):
            for qc in qcs:
                ps = ps_score.tile([P, 1024], F32, tag="score",
                                   name=f"qk_{mc}_{qc}")
                for kc in range(KCH):
                    nc.tensor.matmul(
                        ps[:, 0:512],
                        wqk_sb[:, kc, mc * 128:(mc + 1) * 128],
                        xt_sb[:, kc, qc * 512:(qc + 1) * 512],
                        start=(kc == 0), stop=(kc == KCH - 1))
                nc.vector.tensor_copy(q2_sb[0:D, mc, qc * 512:(qc + 1) * 512],
                                      ps[0:D, 0:512])
                kview = ps[D:P, 0:512].rearrange("p (b c) -> p b c", c=128)
                k2w = k2_sb[:, mc, qc * 256:(qc + 1) * 256].rearrange(
                    "p (b c) -> p b c", c=128)
                nc.vector.tensor_copy(k2w[0:D], kview[:, 0::2, :])
                nc.vector.tensor_copy(k2w[D:P], kview[:, 1::2, :])
                if dup_now:
                    nc.vector.tensor_copy(
                        q2_sb[D:P, mc, qc * 512:(qc + 1) * 512],
                        q2_sb[0:D, mc, qc * 512:(qc + 1) * 512])
            if not dup_now and qcs[-1] == QC - 1:
                # duplicate q into the lower half (DVE 4x bf16 copy)
                nc.vector.tensor_copy(q2_sb[D:P, mc, :], q2_sb[0:D, mc, :])

        def v_proj(ts):
            if ts[0] == 0:
                nc.vector.memset(v_sb[:, :, :, D:D + 1], 1.0)
            for t in ts:
                ps = ps_score.tile([P, 1024], F32, tag="score", name=f"v_{t}")
                for kc in range(KCH):
                    nc.tensor.matmul(
                        ps[:, 0:HPC * D],
                        xt_sb[:, kc, t * 128:(t + 1) * 128],
                        wv_sb[:, kc, :],
                        start=(kc == 0), stop=(kc == KCH - 1))
                nc.vector.tensor_copy(
                    v_sb[:, t, :, 0:D],
                    ps[:, 0:HPC * D].rearrange("p (h d) -> p h d", h=HPC))

        e_tiles = {}
        warm_state = {"pvs": None, "i": 0}

        def warm_pulse(e2, qq):
            # HAM warmkeeper: a K=1 N=512 matmul (~213ns) gated on an exp
            # of the NEXT pair -- fires at the same semaphore points the
            # following step's score matmuls wait on, filling the PE's
            # trailing idle without delaying any real work.
            pvs = warm_state["pvs"]
            if pvs is None:
                return
            i = warm_state["i"] = (warm_state["i"] + 1) % QC
            nc.tensor.matmul(pvs[i][96:97, :],
                             e2[0:1, qq, 0, 0:1], e2[0:1, qq, 0, :],
                             start=True, stop=True, skip_group_check=True,
                             tile_position=(0, 96))

        def scores_exp_pair(h, j, qqs=None):
            # t-chunks (2j, 2j+1) as concurrent PE row-tiles: even chunk from
            # k2 rows 0:64 -> psum cols 0:512, odd from rows 64:128 -> cols
            # 512:1024. One exp per psum tile keeps the pair on one sem, so
            # the two matmuls issue back-to-back and overlap in the array.
            # e2 dims [qq, parity, tok] keep the exp output AP contiguous.
            if qqs is None:
                qqs = range(QC)
            if (h, 2 * j) in e_tiles:
                e2 = e_tiles[(h, 2 * j)]
            else:
                e2 = epool.tile([P, QC, 2, 512], BF16, tag="E",
                                name=f"e_{h}_{j}")
                e_tiles[(h, 2 * j)] = e2
            for qq in qqs:
                ps = ps_score.tile([P, 1024], F32, tag="score", name="sc")
                qsl = slice(qq * 512, (qq + 1) * 512)
                nc.tensor.matmul(
                    ps[:, 0:512],
                    k2_sb[0:D, h, j * 128:(j + 1) * 128],
                    q2_sb[0:D, h, qsl], start=True, stop=True)
                nc.tensor.matmul(
                    ps[:, 512:1024],
                    k2_sb[D:P, h, j * 128:(j + 1) * 128],
                    q2_sb[D:P, h, qsl], start=True, stop=True)
                nc.scalar.activation(
                    e2[:, qq, :, :], ps[:],
                    mybir.ActivationFunctionType.Exp, scale=esc)

        def pv_chunk(h, t, pvs):
            e2 = e_tiles[(h, t & ~1)]
            for qc in range(QC):
                nc.tensor.matmul(
                    pvs[qc][0:D + 1, :],
                    v_sb[:, t, h, :],
                    e2[:, qc, t & 1, :],
                    start=(t == 0), stop=(t == TC - 1))
            if t & 1:
                e_tiles.pop((h, t & ~1))

        # sumexp rows gathered at 32-aligned partitions -> one batched
        # reciprocal; 1/Z broadcast via K=1 matmuls at row positions
        # 0/32/64/96. Stage a (copies + reciprocal) releases the PV
        # accumulators; stage b (rb matmuls + mults) is emitted two pairs
        # later so the PE never FIFO-blocks on the reciprocal. Head 2's
        # staging runs on both ACT (idle after the last exp) and DVE.
        def norm_stage_a(h, pvs):
            if h == HPC - 1:
                def stage_copy(dst, src, qc):
                    if qc < 2:
                        nc.vector.tensor_copy(dst, src)
                    else:
                        nc.scalar.copy(dst, src)
            else:
                def stage_copy(dst, src, qc):
                    nc.vector.tensor_copy(dst, src)
            rt = spool.tile([97, 512], F32, tag="rt")
            nc.vector.memset(rt[:], 1.0)
            pvsbs = []
            for qc in range(QC):
                pvsb = pvpool.tile([D, 512], F32, tag="pvsb",
                                   name=f"pvsb_{h}_{qc}")
                stage_copy(pvsb[:], pvs[qc][0:D, :], qc)
                stage_copy(rt[32 * qc:32 * qc + 1, :],
                           pvs[qc][D:D + 1, :], qc)
                pvsbs.append(pvsb)
            rr = spool.tile([97, 512], F32, tag="rr")
            if h == HPC - 1:
                # split so warmkeeper matmuls can stagger through it
                nc.vector.reciprocal(rr[:, 0:256], rt[:, 0:256])
                nc.vector.reciprocal(rr[:, 256:512], rt[:, 256:512])
            else:
                nc.vector.reciprocal(rr[:], rt[:])
            return rr, pvsbs

        def norm_stage_b(h, rr, pvsbs, pool=None):
            for qc in range(QC):
                if pool is None:
                    rb = ps_score.tile([P, 1024], F32, tag="score",
                                       name=f"rb_{h}_{qc}")
                else:
                    rb = pool.tile([P, 512], F32, tag="pv",
                                   name=f"rb_{h}_{qc}")
                nc.tensor.matmul(rb[0:D, 0:512],
                                 ones4[32 * qc:32 * qc + 1, :],
                                 rr[32 * qc:32 * qc + 1, :],
                                 start=True, stop=True,
                                 tile_position=(32 * qc, 0))
                qsl = slice(qc * 512, (qc + 1) * 512)
                dst = (aT_0[:, qsl] if h == 0
                       else aT_bc[(h - 1) * D:h * D, qsl])
                nc.vector.tensor_tensor(dst, pvsbs[qc][:], rb[0:D, 0:512],
                                        mybir.AluOpType.mult)

        # Per-head output projection: head hh's 64 aT rows x its 64 rows of
        # w_proj (K=64 matmuls). Head 0 runs during window 1, head 1 during
        # late window 2 (both only need their own norm), head 2 in the tail.
        # Staging converts to bf16; 4 chunks batch into one DMA issue.
        ostage_cur = {}

        def proj_head(hh, sc, use_act=False):
            # hh=0: head-0 rows only (runs during w1, stream out0)
            # hh=2: heads 1+2 accumulated in one psum (tail, stream out1)
            s_sl = slice(sc * 128, (sc + 1) * 128)
            if hh == 2 and sc % 4 >= 2:
                ta = ps_pv.tile([P, 512], F32, tag="pv", name=f"pja_{sc}")
                tb = ps_pv.tile([P, 512], F32, tag="pv", name=f"pjb_{sc}")
                pa, pb = ta[:], tb[:, 0:256]
            else:
                ps = ps_score.tile([P, 1024], F32, tag="score",
                                   name=f"pj_{hh}_{sc}")
                pa, pb = ps[:, 0:512], ps[:, 512:768]
            if hh == 0:
                nc.tensor.matmul(pa, aT_0[:, s_sl], wp0_sb[:, 0:512],
                                 start=True, stop=True)
                nc.tensor.matmul(pb, aT_0[:, s_sl], wp0_sb[:, 512:768],
                                 start=True, stop=True)
                ost_idx = 0
            else:
                nc.tensor.matmul(pa, aT_bc[:, s_sl], wp12_sb[:, 0:512],
                                 start=True, stop=True)
                nc.tensor.matmul(pb, aT_bc[:, s_sl], wp12_sb[:, 512:768],
                                 start=True, stop=True)
                ost_idx = 1
            if sc % 4 == 0:
                ostage_cur[ost_idx] = opool.tile([P, 4, NX], BF16, tag="o",
                                                 name=f"os_{hh}_{sc}")
            ostage = ostage_cur[ost_idx]
            nc.vector.tensor_copy(ostage[:, sc % 4, 0:512], pa)
            if use_act:
                nc.scalar.copy(ostage[:, sc % 4, 512:768], pb)
            else:
                nc.vector.tensor_copy(ostage[:, sc % 4, 512:768], pb)
            if sc % 4 == 3:
                orows = outs_d[ost_idx][(sc - 3) * 128:(sc + 1) * 128, :]
                nc.sync.dma_start(
                    orows.rearrange("(c p) n -> p c n", p=P), ostage[:])

        # ---- emission order = per-engine FIFO order = pipeline order ----
        # One long exp stream (24 t-chunk pairs across 3 heads). Per step k:
        # the NEXT pair's score tiles, 1:1-interleaved with filler units
        # (v_proj / later-head qk_proj, each a ps_score alloc that trickles
        # with the exp-paced pool rotation), then PV for pair k-2 (its exps
        # long done -> a dependency-free dense burst that keeps the PE warm).
        # Head-2 PV runs a catch-up schedule so the last pair is JIT and the
        # tail starts right after the last exp. Norms are deferred two pairs
        # into the next window and MUST be emitted before the next head's
        # accumulators are allocated (pool-slot reuse).
        for qc in range(QC):
            qk_proj(0, [qc], dup_now=True)
            scores_exp_pair(0, 0, qqs=[qc])

        def vu(t):
            return lambda: v_proj([t])

        def qku(mc, qc):
            return lambda: qk_proj(mc, [qc], dup_now=True)

        def pu(hh, sc):
            return lambda: proj_head(hh, sc)

        unit_sched = {
            0: [vu(0), vu(1), vu(2), vu(3)],
            1: [vu(4), vu(5), vu(6), vu(7)],
            2: [vu(8), vu(9), vu(10)],
            3: [vu(11), vu(12), vu(13)],
            4: [vu(14), vu(15), qku(1, 0)],
            5: [qku(1, 1), qku(1, 2)],
            6: [qku(1, 3)],
            8: [qku(2, 0), qku(2, 1)],
            9: [qku(2, 2), qku(2, 3)],
        }
        # head-0 projection during w1/w2 (needs only norm(0), done at k=11)
        for k in range(12, 20):
            unit_sched[k] = [pu(0, 2 * (k - 12)), pu(0, 2 * (k - 12) + 1)]
        pv_sched = {k: [k - 2] for k in range(2, 18)}
        pv_sched.update({18: [16, 17], 19: [18], 20: [19, 20],
                         21: [21], 22: [22], 23: [23]})
        norm_a_sched = {9: 0, 17: 1}   # after PV pair 8h+7 lands (lag-2)
        norm_b_sched = {11: 0, 19: 1}  # two steps later: recip already done
        NK = HPC * NPAIR
        pvs_store = {}
        norm_st = {}

        for k in range(NK):
            us = list(unit_sched.get(k, []))
            if k + 1 < NK:
                h2, j2 = divmod(k + 1, NPAIR)
                for qq in range(QC):
                    scores_exp_pair(h2, j2, qqs=[qq])
                    if us:
                        us.pop(0)()
            for u in us:
                u()
            if k in norm_b_sched:
                hn = norm_b_sched[k]
                norm_stage_b(hn, *norm_st.pop(hn))
            for p in pv_sched.get(k, []):
                ph, pj = divmod(p, NPAIR)
                if pj == 0:
                    pvs_store[ph] = [ps_pv.tile([P, 512], F32, tag="pv",
                                                name=f"pv_{ph}_{qc}")
                                     for qc in range(QC)]
                    warm_state["pvs"] = pvs_store[ph]
                pv_chunk(ph, 2 * pj, pvs_store[ph])
                pv_chunk(ph, 2 * pj + 1, pvs_store[ph])
            if k in norm_a_sched:
                hn = norm_a_sched[k]
                norm_st[hn] = norm_stage_a(hn, pvs_store.pop(hn))


        # ---- tail: norm(2) + head-2 output projection ----
        pvs2 = pvs_store.pop(2)
        rr2, pvsbs2 = norm_stage_a(2, pvs2)
        # warmkeeper matmuls staggered on the staging copies / reciprocal
        # halves so the PE keeps HAM busy through the ~7us norm window
        for i, dep in enumerate(pvsbs2 + [rr2, rr2]):
            dmy = ps_pv.tile([P, 512], F32, tag="pv", name=f"dmy_{i}")
            if i < QC:
                src = dep[0:1, 0:512]
            elif i == QC:
                src = dep[0:1, 0:256]
            else:
                src = dep[0:1, 256:512]
            nc.tensor.matmul(dmy[0:1, 0:256 if i >= QC else 512],
                             src[:, 0:1], src, start=True, stop=True)
        norm_stage_b(2, rr2, pvsbs2, pool=ps_pv)
        for sc in range(S // 128):
            proj_head(2, sc, use_act=True)


# ---------------------------------------------------------------------------
# host side
# ---------------------------------------------------------------------------

def fp8_scales(hidden, w_attn):
    """Kept for interface compatibility; bf16 path needs no scaling."""
    return 1.0, 1.0


def make_in_maps(hidden_states, w_attn, b_attn, w_proj, S=2048,
                 use_bias=False):
    """Build the 8 per-core input dicts (numpy bf16)."""
    bf = ml_dtypes.bfloat16
    KCH = 7 if use_bias else 6
    KDIM = KCH * 128
    hidden = np.asarray(hidden_states)
    w_attn = np.asarray(w_attn)
    b_attn = np.asarray(b_attn)
    w_proj = np.asarray(w_proj)

    xts = []
    for b in range(hidden.shape[0]):
        xt = np.zeros((KDIM, S), dtype=bf)
        xt[0:NX, :] = hidden[b].T.astype(bf)
        if use_bias:
            xt[NX, :] = 1.0
        # partition-major layout: [p, qc, c, tok]
        xt = np.ascontiguousarray(
            xt.reshape(KCH, 128, S // 512, 512).transpose(1, 2, 0, 3))
        xts.append(xt)

    in_maps = []
    for c in range(N_CORES):
        b = c // (N_CORES // hidden.shape[0])
        h0 = HPC * (c % (N_CORES // hidden.shape[0]))
        wqk = np.zeros((KDIM, 6 * D), dtype=bf)
        wv = np.zeros((KDIM, HPC * D), dtype=bf)
        for i in range(HPC):
            h = h0 + i
            wqk[0:NX, (2 * i) * D:(2 * i + 1) * D] = \
                w_attn[:, h * D:(h + 1) * D].astype(bf)
            wqk[0:NX, (2 * i + 1) * D:(2 * i + 2) * D] = \
                w_attn[:, NX + h * D:NX + (h + 1) * D].astype(bf)
            wv[0:NX, i * D:(i + 1) * D] = \
                w_attn[:, 2 * NX + h * D:2 * NX + (h + 1) * D].astype(bf)
            if use_bias:
                wqk[NX, (2 * i) * D:(2 * i + 1) * D] = \
                    b_attn[h * D:(h + 1) * D].astype(bf)
                wqk[NX, (2 * i + 1) * D:(2 * i + 2) * D] = \
                    b_attn[NX + h * D:NX + (h + 1) * D].astype(bf)
                wv[NX, i * D:(i + 1) * D] = \
                    b_attn[2 * NX + h * D:2 * NX + (h + 1) * D].astype(bf)
        wp = w_proj[h0 * D:(h0 + HPC) * D, :].astype(bf)
        in_maps.append({"xt": xts[b], "wqk": wqk, "wv": wv, "wp": wp})
    return in_maps


_CACHE = {}


def kernel(hidden_states, w_attn, b_attn, w_proj, b_proj):
    from concourse.bass_utils import run_bass_kernel_spmd

    hidden = np.asarray(hidden_states, dtype=np.float32)
    B, S, _ = hidden.shape
    use_bias = bool(np.any(np.asarray(b_attn)))
    xs, ws = fp8_scales(hidden, np.asarray(w_attn, dtype=np.float32))
    in_maps = make_in_maps(hidden, w_attn, b_attn, w_proj, S=S,
                           use_bias=use_bias)

    esc = 0.125 / (xs * ws) ** 2
    key = (S, use_bias, esc)
    if key not in _CACHE:
        _CACHE[key] = build_nc(S=S, use_bias=use_bias, esc=esc)
    nc = _CACHE[key]

    res = run_bass_kernel_spmd(nc, in_maps, core_ids=list(range(N_CORES)))
    out = gather_out(res.results, B, S, xs * ws)
    out += np.asarray(b_proj, dtype=np.float32)
    return out


def gather_out(results, B, S, scale=1.0):
    cpb = N_CORES // B
    out = np.zeros((B, S, NX), dtype=np.float32)
    for c in range(N_CORES):
        for h in range(2):
            out[c // cpb] += np.asarray(results[c][f"out{h}"],
                                        dtype=np.float32)
    out /= scale
    return out
